# revision 8
# baseline (speedup 1.0000x reference)
"""RNN-T Joiner kernel for 8 Trainium2 NeuronCores.

out[b,t,u,:] = tanh(enc[b,t,:] + pred[b,u,:]) @ W.T + b

Sharding: 2 cores per batch, each takes half the t range (200 t), so every
core owns one batch and 20000 (t,u) cells. Data path is bf16 end-to-end
(enc/pred/W/logits/output) with f32 PSUM accumulation; the 2e-2 rel-err
budget dwarfs the ~0.5% bf16 error.

Per core engine split (budgeted against the PE's ~137us of matmuls):
  - producers (logit = enc[c,t] + pred[c,u], broadcast add): DVE runs at 1x
    on broadcast APs, so units are spread over DVE / GpSimd / fused-ACT
    (per-t tanh with per-partition bias) by a static table.
  - ACT: big in-place tanh per (ck, block)
  - PE: psum[cells, v] += logit[c, cells].T @ W[c, v], 4 chunk matmuls per
    128-cell tile, 4 tiles (banks) per psum group
  - DVE: one bias-add per 4-bank group (psum f32 + bias f32 -> bf16 sbuf)
  - DMA: 512KB bf16 stores per group

Constants (W, f32 bias via bitcast, enc slice, pred) are packed host-side
into one bf16 [128, NCOL] tensor -> a single input DMA.
"""

import sys

sys.path.insert(0, "/opt/trn_rl_repo")

import numpy as np
import ml_dtypes

import concourse.bass as bass
import concourse.bacc as bacc
import concourse.mybir as mybir
from concourse.tile import TileContext
from concourse.bass_utils import run_bass_kernel_spmd

B, T, U, C, V = 4, 400, 100, 512, 512
NCORES = 8
TSC = T // 2  # 200 t per core (2 cores per batch)
P = 128
CK = C // P  # 4 contraction chunks
CELLS = TSC * U  # 20000 cells per core
F32 = mybir.dt.float32
BF16 = mybir.dt.bfloat16
BF = ml_dtypes.bfloat16

# t-blocks: 6 blocks of 32 t (3200 cells = 25 tiles) + 1 block of 8 t
# (800 cells) => 157 matmul tiles of <=128 cells, no tile straddles blocks.
BLOCK_T = [32] * 6 + [8]
BLOCK_CELLS = [bt * U for bt in BLOCK_T]
BLOCK_C0 = np.cumsum([0] + BLOCK_CELLS).tolist()  # cell offset per block
NBLK = len(BLOCK_T)

# mm tiles: (cell_start, m)
TILES = [(s, P) for s in range(0, (CELLS // P) * P, P)]
if CELLS % P:
    TILES.append(((CELLS // P) * P, CELLS % P))
# psum groups of up to 4 tiles (4 banks)
GROUPS = [TILES[i : i + 4] for i in range(0, len(TILES), 4)]

# packed consts layout (bf16 columns)
W_OFF = 0  # [ck, v] -> 4*512
BIAS_OFF = W_OFF + CK * V  # f32 bias replicated [4, 512], stored as 2x bf16
BIASB_OFF = BIAS_OFF + 2 * 4 * V  # bf16 bias replicated [4, 512]
ENC_OFF = BIASB_OFF + 4 * V  # [ck, t] -> 4*200
PRED_OFF = ENC_OFF + CK * TSC  # [ck, u] -> 4*100
NCOL = PRED_OFF + CK * U  # 9392

# producer unit engine assignment: unit = blk*4 + ck. One GpSimd add per
# block (~9us each incl drain; two would serialize past the 23us PE budget
# per block). Rest on DVE; no fused-ACT producers (ACT carries consumer
# copies instead).
GPS_UNITS = {3, 7, 11, 15, 19, 23, 27}
ACTF_UNITS = set()

# consumer path per psum group: most go ACT copy (psum f32 -> osb bf16,
# ~1.96us) + DVE in-place 2x bf16 bias add (~1.22us); a few direct DVE
# tensor_tensor adds (~2.77us) to keep ACT under its tanh+copy budget.
DIRECT_GROUPS = {5, 12, 19, 26, 33, 39}

_cache = {}


def _build():
    nc = bacc.Bacc("TRN2", target_bir_lowering=False, debug=False)
    consts = nc.declare_dram_parameter("consts", [P, NCOL], BF16, isOutput=False)
    out = nc.declare_dram_parameter("out", [TSC, U, V], BF16, isOutput=True)
    ob = out.ap().rearrange("t u v -> (t u) v")  # [20000, 512]

    with TileContext(nc) as tc:
        with (
            tc.tile_pool(name="consts", bufs=1) as cpool,
            tc.tile_pool(name="logit", bufs=2) as logit_pool,
            tc.tile_pool(name="osb", bufs=4) as out_pool,
            tc.tile_pool(name="psum", bufs=2, space="PSUM") as psum_pool,
        ):
            cs = cpool.tile([P, NCOL], BF16, tag="cs")
            nc.sync.dma_start(out=cs, in_=consts.ap())

            wview = cs[:, W_OFF : W_OFF + CK * V].rearrange(
                "p (ck v) -> p ck v", ck=CK
            )
            bias_f32 = cs[:, BIAS_OFF : BIAS_OFF + 2 * 4 * V].bitcast(F32)
            bias_bf = cs[:, BIASB_OFF : BIASB_OFF + 4 * V]
            eview = cs[:, ENC_OFF : ENC_OFF + CK * TSC].rearrange(
                "p (ck t) -> p ck t", ck=CK
            )
            pview = cs[:, PRED_OFF : PRED_OFF + CK * U].rearrange(
                "p (ck u) -> p ck u", ck=CK
            )

            lg = {}  # (blk, ck) -> flat [P, cells] bf16 view

            def emit_producer(blk, ck):
                bt = BLOCK_T[blk]
                t0 = sum(BLOCK_T[:blk])
                ncell = bt * U
                lgt = logit_pool.tile([P, 3200], BF16, tag=f"lg{ck}")
                lg[(blk, ck)] = lgt
                v3 = lgt[:, :ncell].rearrange("p (t u) -> p t u", t=bt)
                unit = blk * 4 + ck
                if unit in ACTF_UNITS:
                    # fused add+tanh, one ACT op per t (bias is per-partition)
                    for t in range(bt):
                        nc.scalar.activation(
                            out=v3[:, t, :],
                            in_=pview[:, ck, :],
                            func=mybir.ActivationFunctionType.Tanh,
                            bias=eview[:, ck, t0 + t : t0 + t + 1],
                        )
                else:
                    e_col = (
                        eview[:, ck, t0 : t0 + bt]
                        .unsqueeze(2)
                        .broadcast_to([P, bt, U])
                    )
                    p_row = (
                        pview[:, ck, :].unsqueeze(1).broadcast_to([P, bt, U])
                    )
                    eng = nc.gpsimd if unit in GPS_UNITS else nc.vector
                    eng.tensor_add(out=v3, in0=e_col, in1=p_row)
                    nc.scalar.activation(
                        out=lgt[:, :ncell],
                        in_=lgt[:, :ncell],
                        func=mybir.ActivationFunctionType.Tanh,
                    )

            def emit_group(g):
                tiles = GROUPS[g]
                ps = psum_pool.tile([P, 4 * V], F32, tag="ps")
                for j, (s, m) in enumerate(tiles):
                    blk = min(s // 3200, NBLK - 1)
                    off = s - BLOCK_C0[blk]
                    for ck in range(CK):
                        nc.tensor.matmul(
                            ps[:m, j * V : (j + 1) * V],
                            lhsT=lg[(blk, ck)][:, off : off + m],
                            rhs=wview[:, ck, :],
                            start=(ck == 0),
                            stop=(ck == CK - 1),
                        )
                ncol = len(tiles) * V
                mlast = tiles[-1][1]
                mrows = P if len(tiles) > 1 else mlast
                osb = out_pool.tile([P, 4 * V], BF16, tag="osb")
                if g in DIRECT_GROUPS:
                    nc.vector.tensor_add(
                        out=osb[:mrows, :ncol],
                        in0=ps[:mrows, :ncol],
                        in1=bias_f32[:mrows, :ncol],
                    )
                else:
                    nc.scalar.copy(out=osb[:mrows, :ncol], in_=ps[:mrows, :ncol])
                    nc.vector.tensor_add(
                        out=osb[:mrows, :ncol],
                        in0=osb[:mrows, :ncol],
                        in1=bias_bf[:mrows, :ncol],
                    )
                # full 128-cell tiles in one strided DMA; ragged tail alone
                nfull = len(tiles) - (1 if mlast != P else 0)
                c0 = tiles[0][0]
                if nfull:
                    dst = ob[c0 : c0 + nfull * P, :].rearrange(
                        "(j p) v -> p j v", p=P
                    )
                    src = osb[:, : nfull * V].rearrange(
                        "p (j v) -> p j v", v=V
                    )
                    nc.sync.dma_start(out=dst, in_=src)
                if mlast != P:
                    s, m = tiles[-1]
                    nc.sync.dma_start(
                        out=ob[s : s + m, :],
                        in_=osb[:m, (len(tiles) - 1) * V : ncol],
                    )

            # interleave: emit block k+1's producers before block k's groups
            # so the scheduler keeps producers a block ahead of the PE
            next_g = 0
            for blk in range(NBLK):
                for ck in range(CK):
                    emit_producer(blk, ck)
                done = BLOCK_C0[blk]  # groups fully inside blocks < blk
                while next_g < len(GROUPS) and (
                    GROUPS[next_g][-1][0] + GROUPS[next_g][-1][1] <= done
                ):
                    emit_group(next_g)
                    next_g += 1
            while next_g < len(GROUPS):
                emit_group(next_g)
                next_g += 1
    nc.compile()
    return nc


def _install_ntff_hook():
    """This image's antenv lacks axon_hooks; wire the ctypes NTFF hook from
    trn_boot against the axon PJRT .so so trace=True works."""
    if "antenv.axon_hooks" in sys.modules:
        return
    import types

    holder = [None]
    mod = types.ModuleType("antenv.axon_hooks")
    mod.set_axon_ntff_profile_hook = lambda h: holder.__setitem__(0, h)
    mod.get_axon_ntff_profile_hook = lambda: holder[0]
    sys.modules["antenv.axon_hooks"] = mod
    try:
        sys.path.insert(0, "/root/.axon_site/trn_agent_boot")
        from trn_boot import _ntff_profile_via_ctypes

        mod.set_axon_ntff_profile_hook(
            _ntff_profile_via_ctypes("/opt/axon/libaxon_pjrt.so")
        )
    except Exception as e:  # degrade to no tracing
        print(f"NTFF hook install failed: {e}", file=sys.stderr)


def _run(in_maps, trace=False, tmpdir=None):
    if "nc" not in _cache:
        _cache["nc"] = _build()
    if trace:
        _install_ntff_hook()
    return run_bass_kernel_spmd(
        _cache["nc"], in_maps, list(range(NCORES)), trace=trace, tmpdir=tmpdir
    )


def make_in_maps(encoder_out, predictor_out, W, b):
    encoder_out = np.asarray(encoder_out, dtype=np.float32)
    predictor_out = np.asarray(predictor_out, dtype=np.float32)
    W = np.asarray(W, dtype=np.float32)
    b = np.asarray(b, dtype=np.float32)

    # [p, ck, v] <- W[v, ck*P+p]
    w_pack = W.reshape(V, CK, P).transpose(2, 1, 0).reshape(P, CK * V)
    bias_rep = np.tile(b, (P, 4, 1)).reshape(P, 4 * V).astype(np.float32)
    bias_bf = bias_rep.view(BF)  # [P, 2*4*V] raw f32 bytes as bf16 cols

    in_maps = []
    for i in range(NCORES):
        bb, half = i // 2, i % 2
        base = np.zeros((P, NCOL), BF)
        base[:, W_OFF : W_OFF + CK * V] = w_pack.astype(BF)
        base[:, BIAS_OFF : BIAS_OFF + 2 * 4 * V] = bias_bf
        base[:, BIASB_OFF : BIASB_OFF + 4 * V] = bias_rep.astype(BF)
        enc_s = encoder_out[bb, half * TSC : (half + 1) * TSC, :]  # [t, c]
        base[:, ENC_OFF : ENC_OFF + CK * TSC] = (
            enc_s.reshape(TSC, CK, P).transpose(2, 1, 0).reshape(P, -1)
        ).astype(BF)
        base[:, PRED_OFF : PRED_OFF + CK * U] = (
            predictor_out[bb].reshape(U, CK, P).transpose(2, 1, 0).reshape(P, -1)
        ).astype(BF)
        in_maps.append({"consts": base})
    return in_maps


def gather(results):
    full = np.empty((B, T, U, V), np.float32)
    for i in range(NCORES):
        bb, half = i // 2, i % 2
        full[bb, half * TSC : (half + 1) * TSC] = np.asarray(
            results[i]["out"]
        ).astype(np.float32)
    return full


def kernel(encoder_out, predictor_out, W, b):
    in_maps = make_in_maps(encoder_out, predictor_out, W, b)
    res = _run(in_maps, trace=False)
    return gather(res.results)


# revision 9
# speedup vs baseline: 1.5354x; 1.5354x over previous
"""RNN-T Joiner kernel for 8 Trainium2 NeuronCores.

out[b,t,u,:] = tanh(enc[b,t,:] + pred[b,u,:]) @ W.T + b

Sharding: 2 cores per batch, each takes half the t range (200 t), so every
core owns one batch and 20000 (t,u) cells. Data path is bf16
(enc/pred/W/logits/output) with f32 PSUM accumulation; the 2e-2 rel-err
budget dwarfs the ~0.5% bf16 error.

Per-core engine budget (PE matmuls ~140us are the wall):
  - producers (logit = enc[c,t] + pred[c,u], broadcast add): DVE broadcast
    APs run at 1x (~3.5us/32t block), so one add per block goes to GpSimd
    (~9us incl drain) and four late-block adds run fused on ACT (per-t tanh
    with per-partition enc bias); the rest stay on DVE.
  - ACT: big in-place tanh per (ck, block), ~2.9us
  - PE: psum[cells, v] += logit[c, cells].T @ W[c, v]; 4 chunk matmuls per
    <=128-cell tile, 4 tiles (banks) per psum group, double-buffered groups
  - DVE: one bias-add per group (psum f32 + bias f32 -> bf16 sbuf), ~2.3us
  - DMA: ~512KB bf16 stores per group

The t-blocks open with an 8t micro-block so the PE starts ~8us in instead
of ~30us; the consts DMA is split so enc/pred land before W/bias.
"""

import sys

sys.path.insert(0, "/opt/trn_rl_repo")

import numpy as np
import ml_dtypes

import concourse.bass as bass
import concourse.bacc as bacc
import concourse.mybir as mybir
from concourse.tile import TileContext
from concourse.bass_utils import run_bass_kernel_spmd

B, T, U, C, V = 4, 400, 100, 512, 512
NCORES = 8
TSC = T // 2  # 200 t per core (2 cores per batch)
P = 128
CK = C // P  # 4 contraction chunks
CELLS = TSC * U  # 20000 cells per core
F32 = mybir.dt.float32
BF16 = mybir.dt.bfloat16
BF = ml_dtypes.bfloat16

BLOCK_T = [8, 24, 32, 32, 32, 32, 32, 8]
BLOCK_CELLS = [bt * U for bt in BLOCK_T]
BLOCK_C0 = np.cumsum([0] + BLOCK_CELLS).tolist()
NBLK = len(BLOCK_T)
MAXBC = max(BLOCK_CELLS)

# mm tiles: (blk, local_offset, m, global_cell_start); <=128 cells, within
# one block so each tile reads one logit SBUF tile
TILES = []
for _blk in range(NBLK):
    _c = BLOCK_CELLS[_blk]
    for _s in range(0, _c, P):
        TILES.append((_blk, _s, min(P, _c - _s), BLOCK_C0[_blk] + _s))
GROUPS = [TILES[i : i + 4] for i in range(0, len(TILES), 4)]

# packed consts layout (bf16 columns); enc/pred first so a small leading
# DMA unblocks producers before the W/bias bulk lands
ENC_OFF = 0  # [ck, t] -> 4*200
PRED_OFF = ENC_OFF + CK * TSC  # [ck, u] -> 4*100
W_OFF = PRED_OFF + CK * U  # [ck, v] -> 4*512
BIAS_OFF = W_OFF + CK * V  # f32 bias replicated [4, 512] as 2x bf16 cols
NCOL = BIAS_OFF + 2 * 4 * V  # 7344

# producer unit engines (unit = blk*4 + ck): one GpSimd add per 24/32t
# block, fused-ACT adds only on late blocks (early blocks must fill fast)
GPS_UNITS = {blk * 4 + 3 for blk in range(1, 7)}
ACTF_UNITS = {blk * 4 + 2 for blk in range(4, 7)}

_cache = {}


def _build():
    nc = bacc.Bacc("TRN2", target_bir_lowering=False, debug=False)
    consts = nc.declare_dram_parameter("consts", [P, NCOL], BF16, isOutput=False)
    out = nc.declare_dram_parameter("out", [TSC, U, V], BF16, isOutput=True)
    ob = out.ap().rearrange("t u v -> (t u) v")  # [20000, 512]

    with TileContext(nc) as tc:
        with (
            tc.tile_pool(name="consts", bufs=1) as cpool,
            tc.tile_pool(name="logit", bufs=2) as logit_pool,
            tc.tile_pool(name="osb", bufs=4) as out_pool,
            tc.tile_pool(name="psum", bufs=2, space="PSUM") as psum_pool,
        ):
            cs = cpool.tile([P, NCOL], BF16, tag="cs")
            nc.sync.dma_start(out=cs[:, :W_OFF], in_=consts.ap()[:, :W_OFF])
            nc.sync.dma_start(out=cs[:, W_OFF:], in_=consts.ap()[:, W_OFF:])

            eview = cs[:, ENC_OFF : ENC_OFF + CK * TSC].rearrange(
                "p (ck t) -> p ck t", ck=CK
            )
            pview = cs[:, PRED_OFF : PRED_OFF + CK * U].rearrange(
                "p (ck u) -> p ck u", ck=CK
            )
            wview = cs[:, W_OFF : W_OFF + CK * V].rearrange(
                "p (ck v) -> p ck v", ck=CK
            )
            bias_f32 = cs[:, BIAS_OFF : BIAS_OFF + 2 * 4 * V].bitcast(F32)

            lg = {}  # (blk, ck) -> [P, MAXBC] bf16 tile

            def emit_producer(blk, ck):
                bt = BLOCK_T[blk]
                t0 = sum(BLOCK_T[:blk])
                ncell = bt * U
                lgt = logit_pool.tile([P, MAXBC], BF16, tag=f"lg{ck}")
                lg[(blk, ck)] = lgt
                v3 = lgt[:, :ncell].rearrange("p (t u) -> p t u", t=bt)
                unit = blk * 4 + ck
                if unit in ACTF_UNITS:
                    for t in range(bt):
                        nc.scalar.activation(
                            out=v3[:, t, :],
                            in_=pview[:, ck, :],
                            func=mybir.ActivationFunctionType.Tanh,
                            bias=eview[:, ck, t0 + t : t0 + t + 1],
                        )
                else:
                    e_col = (
                        eview[:, ck, t0 : t0 + bt]
                        .unsqueeze(2)
                        .broadcast_to([P, bt, U])
                    )
                    p_row = (
                        pview[:, ck, :].unsqueeze(1).broadcast_to([P, bt, U])
                    )
                    eng = nc.gpsimd if unit in GPS_UNITS else nc.vector
                    eng.tensor_add(out=v3, in0=e_col, in1=p_row)
                    nc.scalar.activation(
                        out=lgt[:, :ncell],
                        in_=lgt[:, :ncell],
                        func=mybir.ActivationFunctionType.Tanh,
                    )

            def emit_group(g):
                tiles = GROUPS[g]
                ps = psum_pool.tile([P, 4 * V], F32, tag="ps")
                for j, (blk, off, m, _) in enumerate(tiles):
                    for ck in range(CK):
                        nc.tensor.matmul(
                            ps[:m, j * V : (j + 1) * V],
                            lhsT=lg[(blk, ck)][:, off : off + m],
                            rhs=wview[:, ck, :],
                            start=(ck == 0),
                            stop=(ck == CK - 1),
                        )
                ncol = len(tiles) * V
                osb = out_pool.tile([P, 4 * V], BF16, tag="osb")
                nc.vector.tensor_add(
                    out=osb[:, :ncol], in0=ps[:, :ncol], in1=bias_f32[:, :ncol]
                )
                # one DMA per run of full tiles; ragged tiles DMA alone
                j = 0
                while j < len(tiles):
                    if tiles[j][2] == P:
                        j1 = j
                        while j1 < len(tiles) and tiles[j1][2] == P:
                            j1 += 1
                        c0 = tiles[j][3]
                        n = j1 - j
                        dst = ob[c0 : c0 + n * P, :].rearrange(
                            "(k p) v -> p k v", p=P
                        )
                        src = osb[:, j * V : j1 * V].rearrange(
                            "p (k v) -> p k v", v=V
                        )
                        nc.sync.dma_start(out=dst, in_=src)
                        j = j1
                    else:
                        blk, off, m, c0 = tiles[j]
                        nc.sync.dma_start(
                            out=ob[c0 : c0 + m, :],
                            in_=osb[:m, j * V : (j + 1) * V],
                        )
                        j += 1

            # emit groups as soon as the blocks they read are emitted
            next_g = 0
            for blk in range(NBLK):
                for ck in range(CK):
                    emit_producer(blk, ck)
                while next_g < len(GROUPS) and GROUPS[next_g][-1][0] <= blk:
                    emit_group(next_g)
                    next_g += 1
            while next_g < len(GROUPS):
                emit_group(next_g)
                next_g += 1
    nc.compile()
    return nc


def _install_ntff_hook():
    """This image's antenv lacks axon_hooks; wire the ctypes NTFF hook from
    trn_boot against the axon PJRT .so so trace=True works."""
    if "antenv.axon_hooks" in sys.modules:
        return
    import types

    holder = [None]
    mod = types.ModuleType("antenv.axon_hooks")
    mod.set_axon_ntff_profile_hook = lambda h: holder.__setitem__(0, h)
    mod.get_axon_ntff_profile_hook = lambda: holder[0]
    sys.modules["antenv.axon_hooks"] = mod
    try:
        sys.path.insert(0, "/root/.axon_site/trn_agent_boot")
        from trn_boot import _ntff_profile_via_ctypes

        mod.set_axon_ntff_profile_hook(
            _ntff_profile_via_ctypes("/opt/axon/libaxon_pjrt.so")
        )
    except Exception as e:  # degrade to no tracing
        print(f"NTFF hook install failed: {e}", file=sys.stderr)


def _run(in_maps, trace=False, tmpdir=None):
    if "nc" not in _cache:
        _cache["nc"] = _build()
    if trace:
        _install_ntff_hook()
    return run_bass_kernel_spmd(
        _cache["nc"], in_maps, list(range(NCORES)), trace=trace, tmpdir=tmpdir
    )


def make_in_maps(encoder_out, predictor_out, W, b):
    encoder_out = np.asarray(encoder_out, dtype=np.float32)
    predictor_out = np.asarray(predictor_out, dtype=np.float32)
    W = np.asarray(W, dtype=np.float32)
    b = np.asarray(b, dtype=np.float32)

    # [p, ck, v] <- W[v, ck*P+p]
    w_pack = W.reshape(V, CK, P).transpose(2, 1, 0).reshape(P, CK * V)
    bias_rep = np.tile(b, (P, 4, 1)).reshape(P, 4 * V).astype(np.float32)
    bias_bf = bias_rep.view(BF)  # raw f32 bytes as 2x bf16 cols

    in_maps = []
    for i in range(NCORES):
        bb, half = i // 2, i % 2
        base = np.zeros((P, NCOL), BF)
        base[:, W_OFF : W_OFF + CK * V] = w_pack.astype(BF)
        base[:, BIAS_OFF : BIAS_OFF + 2 * 4 * V] = bias_bf
        enc_s = encoder_out[bb, half * TSC : (half + 1) * TSC, :]  # [t, c]
        base[:, ENC_OFF : ENC_OFF + CK * TSC] = (
            enc_s.reshape(TSC, CK, P).transpose(2, 1, 0).reshape(P, -1)
        ).astype(BF)
        base[:, PRED_OFF : PRED_OFF + CK * U] = (
            predictor_out[bb].reshape(U, CK, P).transpose(2, 1, 0).reshape(P, -1)
        ).astype(BF)
        in_maps.append({"consts": base})
    return in_maps


def gather(results):
    full = np.empty((B, T, U, V), np.float32)
    for i in range(NCORES):
        bb, half = i // 2, i % 2
        full[bb, half * TSC : (half + 1) * TSC] = np.asarray(
            results[i]["out"]
        ).astype(np.float32)
    return full


def kernel(encoder_out, predictor_out, W, b):
    in_maps = make_in_maps(encoder_out, predictor_out, W, b)
    res = _run(in_maps, trace=False)
    return gather(res.results)
